# revision 16
# baseline (speedup 1.0000x reference)
"""Trainium2 Bass kernel for a causal single-head attention block.

Reference computation (fp32):
    q = x @ Wq; k = x @ Wk; v = x @ Wv        x: [B=256, T=256, C=384], W*: [384, 64]
    wei = softmax(causal_mask(q @ k.T / sqrt(C)))
    out = wei @ v                              out: [256, 256, 64]

Strategy: pure data parallel over B across 8 NeuronCores (32 batches/core).

All matmul operands are fp16 (full-rate PE streaming, fast weight load);
PSUM accumulation stays fp32.

v2 design vs the earlier kernel: all layout shuffles moved off-device.
  - x is pre-transposed on the HOST to [b, p, (cc t)] so the kernel never
    runs a PE transpose or DVE reshuffle for the input: one contiguous
    1.5KB-per-partition DMA yields xT [C-chunk on partitions, t free].
  - The attention-weighted sum runs in the NATURAL orientation
    (P as stationary, v as moving operand), so the output lands as
    [t on partitions, h] with the softmax denominator Z in an extra
    column (ones appended to v). Normalization is a [128,1] reciprocal +
    per-partition ACT scale -- no final PE transpose, no oe copy.
  - Causal skipping: the (s in [128,256), t in [0,128)) quadrant is fully
    masked, so its scores / exp / oe matmuls are skipped entirely.

Per-batch dataflow:
    xT      = one DMA               [128, 768] fp16 (3 C-chunks of [128, 256])
    qkT     = [Wq|Wk].T @ xT        3 accumulating MMs, M=128 packed (q rows
                                    0-63, k rows 64-127), N=256
    kT      -> own tile             SBUF->SBUF DMA (cross-partition move)
    v       = xT.T @ Wv             6 MMs N=64 (stationary = xT chunks),
                                    copied to v_ext [128, 130] = [v0|1|v1|1]
    sT      = kT.T @ qT             st0: N=256; st1: N=128 (causal skip)
    E       = exp(sT * 1/sqrt(C))   ACT from psum; DVE 0/1-mask on the two
                                    diagonal blocks only (one [128,128] mask)
    O       = P.T @ [v|1]           3 MMs N=65 (stationary = P blocks);
                                    col 64 of each t-tile = denominator Z
    out     = O * (1/Z)             DVE strided reciprocal, ACT per-partition
                                    scale -> [128, 128] fp16, one DMA out

Software pipeline: projections run TWO iterations ahead of their scores
matmul (kT DMA slack); PE stream per iteration:
    sc(b) qkT(b+2) v(b+2) oe(b)
so batch b's softmax chain (ACT exp + DVE mask) hides under batch b+2's
projections.
"""

import numpy as np

N_EMBED = 384
HEAD_SIZE = 64
T = 256
B = 256
N_CORES = 8
B_SHARD = B // N_CORES  # 32
CC = N_EMBED // 128  # 3 contraction chunks
INV_SQRT_C = 1.0 / float(np.sqrt(N_EMBED))

_CACHE = {}

# test.py can flip these before calling kernel()
TRACE = False
LAST_RESULTS = None


def _build_program():
    import concourse.bacc as bacc
    import concourse.mybir as mybir
    import concourse.tile as tile
    from concourse import bass

    f32 = mybir.dt.float32
    f16 = mybir.dt.float16
    ts = bass.ts
    Exp = mybir.ActivationFunctionType.Exp
    Copy = mybir.ActivationFunctionType.Copy

    nc = bacc.Bacc("TRN2", target_bir_lowering=False, debug=False,
                   enable_asserts=False)

    x_d = nc.dram_tensor("x", [B_SHARD, 128, CC * T], f16, kind="ExternalInput")
    wqk_d = nc.dram_tensor("Wqk", [CC, 128, 128], f16, kind="ExternalInput")
    wv_d = nc.dram_tensor("Wv", [CC, 128, HEAD_SIZE], f16, kind="ExternalInput")
    mask_d = nc.dram_tensor("mask01", [128, 128], f16, kind="ExternalInput")
    out_d = nc.dram_tensor("out", [B_SHARD, 128, 2, HEAD_SIZE], f16,
                           kind="ExternalOutput")

    x_ap = x_d.ap()
    out_ap = out_d.ap()

    with tile.TileContext(nc) as tc:
        with (
            tc.tile_pool(name="const", bufs=1) as cpool,
            tc.tile_pool(name="xin", bufs=6) as xin_pool,
            tc.tile_pool(name="proj", bufs=4) as proj_pool,
            tc.tile_pool(name="vxp", bufs=6) as vx_pool,
            tc.tile_pool(name="soft", bufs=5) as soft_pool,
            tc.tile_pool(name="outp", bufs=4) as out_pool,
            tc.tile_pool(name="ps_qk", bufs=2, space="PSUM") as psqk_pool,
            tc.tile_pool(name="ps_v", bufs=2, space="PSUM") as psv_pool,
            tc.tile_pool(name="ps_sc", bufs=2, space="PSUM") as pssc_pool,
            tc.tile_pool(name="ps_oe", bufs=2, space="PSUM") as psoe_pool,
        ):
            # ---- constants ----
            wqk_sb, wv_sb = [], []
            for cc in range(CC):
                t_ = cpool.tile([128, 128], f16, tag=f"wqk{cc}")
                nc.sync.dma_start(t_[:], wqk_d.ap()[cc])
                wqk_sb.append(t_)
                t_ = cpool.tile([128, HEAD_SIZE], f16, tag=f"wv{cc}")
                nc.sync.dma_start(t_[:], wv_d.ap()[cc])
                wv_sb.append(t_)
            mask_sb = cpool.tile([128, 128], f16, tag="mask")
            nc.sync.dma_start(mask_sb[:], mask_d.ap())

            def load_x(b):
                # whole batch in one DMA: contiguous 1536B per partition
                t_ = xin_pool.tile([128, CC * T], f16, tag="xt")
                nc.sync.dma_start(t_[:], x_ap[b])
                return t_

            def proj(xt):
                """q,k,v projections for one batch.
                Returns (qk_sb [128,256] f16, kT [64,256] f16, v_ext [128,130] f16)."""
                ps = psqk_pool.tile([128, T], f32, tag="ps_qk")
                for cc in range(CC):
                    nc.tensor.matmul(ps[:], wqk_sb[cc][:], xt[:, ts(cc, T)],
                                     start=(cc == 0), stop=(cc == CC - 1))
                psv = psv_pool.tile([128, 2 * HEAD_SIZE], f32, tag="ps_v")
                for tt in range(2):
                    for cc in range(CC):
                        nc.tensor.matmul(psv[:, ts(tt, HEAD_SIZE)],
                                         xt[:, cc * T + tt * 128:
                                            cc * T + (tt + 1) * 128],
                                         wv_sb[cc][:],
                                         start=(cc == 0), stop=(cc == CC - 1))
                qk = proj_pool.tile([128, T], f16, tag="qk")
                nc.vector.tensor_copy(qk[:], ps[:])
                kT = proj_pool.tile([HEAD_SIZE, T], f16, tag="kT")
                nc.sync.dma_start(kT[:], qk[HEAD_SIZE:128, :])
                # v_ext = [v_t0 | 1 | v_t1 | 1]: ones columns at 64 and 129.
                # One strided DVE copy moves both v halves (GPSIMD can't read
                # PSUM on TRN2); GPSIMD writes the ones columns.
                vx = vx_pool.tile([128, 130], f16, tag="vx")
                nc.vector.tensor_copy(
                    vx[:].rearrange("p (g h) -> p g h", h=65)[:, :, 0:HEAD_SIZE],
                    psv[:].rearrange("p (g h) -> p g h", h=HEAD_SIZE))
                nc.gpsimd.memset(vx[:, HEAD_SIZE::65], 1.0)
                return qk, kT, vx

            def scores(qk, kT):
                """scoresT psum [128, 384]: st0 at [:, 0:256], st1 (t>=128 only)
                at [:, 256:384]."""
                ps = pssc_pool.tile([128, 384], f32, tag="ps_sc")
                nc.tensor.matmul(ps[:, 0:T], kT[:, 0:128], qk[:HEAD_SIZE, :],
                                 start=True, stop=True)
                nc.tensor.matmul(ps[:, T:384], kT[:, 128:T],
                                 qk[:HEAD_SIZE, 128:T], start=True, stop=True)
                return ps

            def softmax(sc_ps):
                """e [128,384] = exp of all three score blocks in ONE ACT op
                (ACT costs (N+352)/1.2 ns -- fixed overhead dominates small
                ops). pm [128,256]: causal-masked diagonal blocks (GpSimd,
                which is otherwise idle; DVE is loaded)."""
                e = soft_pool.tile([128, 384], f16, tag="e")
                nc.scalar.activation(e[:], sc_ps[:], Exp, scale=INV_SQRT_C)
                pm = soft_pool.tile([128, 256], f16, tag="pm")
                nc.gpsimd.tensor_mul(pm[:, 0:128], e[:, 0:128], mask_sb[:])
                nc.gpsimd.tensor_mul(pm[:, 128:256], e[:, 256:384], mask_sb[:])
                return e, pm

            def oe(e, pm, vx):
                """O psum [128, 130]: t-tile tt at [:, tt*65 : tt*65+65];
                col tt*65+64 = denominator Z."""
                ps = psoe_pool.tile([128, 130], f32, tag="ps_oe")
                nc.tensor.matmul(ps[:, 0:65], pm[:, 0:128], vx[:, 0:65],
                                 start=True, stop=True)
                nc.tensor.matmul(ps[:, 65:130], e[:, 128:T], vx[:, 0:65],
                                 start=True, stop=False)
                nc.tensor.matmul(ps[:, 65:130], pm[:, 128:256], vx[:, 65:130],
                                 start=False, stop=True)
                return ps

            def norm_store(b, ps):
                rz = out_pool.tile([128, 2], f32, tag="rz")
                # both Z columns (offsets 64 and 129) in one strided reciprocal
                nc.vector.reciprocal(rz[:], ps[:, HEAD_SIZE::65])
                o = out_pool.tile([128, 2 * HEAD_SIZE], f16, tag="o")
                # one DVE multiply for both t-tiles: [128, 2, 64] with the
                # per-(partition, tile) reciprocal broadcast along h
                nc.vector.tensor_mul(
                    o[:].rearrange("p (g h) -> p g h", h=HEAD_SIZE),
                    ps[:].rearrange("p (g h) -> p g h", h=65)[:, :, 0:HEAD_SIZE],
                    rz[:].unsqueeze(2).broadcast_to([128, 2, HEAD_SIZE]))
                nc.sync.dma_start(
                    out_ap[b].rearrange("p tt h -> p (tt h)"), o[:])

            # ---- software-pipelined batch loop ----
            # Projections run TWO iterations ahead of their scores matmul so
            # the cross-partition kT DMA has a full iteration of slack, and
            # oe/norm run TWO iterations behind their softmax so NO PE
            # instruction ever waits on the ACT-exp -> GpSimd-mask chain
            # (PE stalls de-warm the HAM clock gate, halving the PE clock).
            # PE stream per iteration: oe(b-2) sc(b) qkT(b+2) v(b+2)
            x_nat = [None] * B_SHARD
            pr, pend = {}, {}
            for j in range(min(4, B_SHARD)):
                x_nat[j] = load_x(j)
            for j in range(min(2, B_SHARD)):
                pr[j] = proj(x_nat[j])
            for b in range(B_SHARD + 2):
                if b + 4 < B_SHARD:
                    x_nat[b + 4] = load_x(b + 4)
                if b >= 2:
                    norm_store(b - 2, oe(*pend.pop(b - 2)))
                if b < B_SHARD:
                    qk, kT, vx = pr.pop(b)
                    sc_ps = scores(qk, kT)
                    e, pm = softmax(sc_ps)
                    if b + 2 < B_SHARD:
                        pr[b + 2] = proj(x_nat[b + 2])
                    pend[b] = (e, pm, vx)

    nc.compile()
    return nc


def _consts():
    # mask01[s, t] = 1 where s <= t else 0 -- shared by both diagonal blocks
    s = np.arange(128)[:, None]
    t = np.arange(128)[None, :]
    return (s <= t).astype(np.float16)


def _spot_check(out, x, Wq, Wk, Wv, batches):
    """Numpy reference for a few batches -- guards against transient device
    flakiness. The fp16 kernel's error is ~3e-3 abs; garbage is ~1e0."""
    for b in batches:
        xb = np.asarray(x[b], dtype=np.float32)
        q = xb @ Wq
        k = xb @ Wk
        v = xb @ Wv
        s = (q @ k.T) * np.float32(INV_SQRT_C)
        tmask = np.tril(np.ones((T, T), dtype=bool))
        s = np.where(tmask, s, -np.inf)
        w = np.exp(s - s.max(axis=-1, keepdims=True))
        o = (w @ v) / w.sum(axis=-1, keepdims=True)
        if np.max(np.abs(out[b] - o)) > 0.05 * max(np.max(np.abs(o)), 1e-3):
            return False
    return True


def kernel(x, Wq, Wk, Wv):
    global LAST_RESULTS
    from concourse import bass_utils

    if "nc" not in _CACHE:
        _CACHE["nc"] = _build_program()
    nc = _CACHE["nc"]

    # host-side layout prep (free): xT[b, p, cc, t] = x[b, t, cc*128 + p]
    x16 = np.asarray(x, dtype=np.float16)
    xt = np.ascontiguousarray(
        x16.transpose(0, 2, 1)               # [B, C, T]
           .reshape(B, CC, 128, T)
           .transpose(0, 2, 1, 3)            # [B, 128, CC, T]
           .reshape(B, 128, CC * T))
    # [Wq | Wk] stacked on the output dim, chunked along the contraction dim
    wqk = np.concatenate([np.asarray(Wq), np.asarray(Wk)], axis=1)
    wqk16 = np.ascontiguousarray(
        wqk.reshape(CC, 128, 2 * HEAD_SIZE), dtype=np.float16)
    wv16 = np.ascontiguousarray(
        np.asarray(Wv, dtype=np.float16).reshape(CC, 128, HEAD_SIZE))
    mask01 = _consts()

    in_maps = []
    for c in range(N_CORES):
        in_maps.append({
            "x": xt[c * B_SHARD:(c + 1) * B_SHARD],
            "Wqk": wqk16, "Wv": wv16, "mask01": mask01,
        })

    xf = np.ascontiguousarray(x, dtype=np.float32)
    Wqf = np.asarray(Wq, dtype=np.float32)
    Wkf = np.asarray(Wk, dtype=np.float32)
    Wvf = np.asarray(Wv, dtype=np.float32)
    check_batches = [c * B_SHARD for c in range(N_CORES)]
    for attempt in range(3):
        res = bass_utils.run_bass_kernel_spmd(
            nc, in_maps, core_ids=list(range(N_CORES)), trace=TRACE)
        LAST_RESULTS = res
        out = np.concatenate(
            [res.results[c]["out"].transpose(0, 2, 1, 3)
             .reshape(B_SHARD, T, HEAD_SIZE) for c in range(N_CORES)], axis=0)
        out = np.ascontiguousarray(out, dtype=np.float32)
        if _spot_check(out, xf, Wqf, Wkf, Wvf, check_batches):
            return out
    return out


# revision 22
# speedup vs baseline: 1.3053x; 1.3053x over previous
"""Trainium2 Bass kernel for a causal single-head attention block.

Reference computation (fp32):
    q = x @ Wq; k = x @ Wk; v = x @ Wv        x: [B=256, T=256, C=384], W*: [384, 64]
    wei = softmax(causal_mask(q @ k.T / sqrt(C)))
    out = wei @ v                              out: [256, 256, 64]

Strategy: pure data parallel over B across 8 NeuronCores (32 batches/core).

All matmul operands are fp16 (full-rate PE streaming, fast weight load);
PSUM accumulation stays fp32.

v2 design vs the earlier kernel: all layout shuffles moved off-device.
  - x is pre-transposed on the HOST to [b, p, (cc t)] so the kernel never
    runs a PE transpose or DVE reshuffle for the input: one contiguous
    1.5KB-per-partition DMA yields xT [C-chunk on partitions, t free].
  - The attention-weighted sum runs in the NATURAL orientation
    (P as stationary, v as moving operand), so the output lands as
    [t on partitions, h] with the softmax denominator Z in an extra
    column (ones appended to v). Normalization is a [128,1] reciprocal +
    per-partition ACT scale -- no final PE transpose, no oe copy.
  - Causal skipping: the (s in [128,256), t in [0,128)) quadrant is fully
    masked, so its scores / exp / oe matmuls are skipped entirely.

Per-batch dataflow:
    xT      = one DMA               [128, 768] fp16 (3 C-chunks of [128, 256])
    qkT     = [Wq|Wk].T @ xT        3 accumulating MMs, M=128 packed (q rows
                                    0-63, k rows 64-127), N=256
    kT      -> own tile             SBUF->SBUF DMA (cross-partition move)
    v       = xT.T @ Wv             6 MMs N=64 (stationary = xT chunks),
                                    copied to v_ext [128, 130] = [v0|1|v1|1]
    sT      = kT.T @ qT             st0: N=256; st1: N=128 (causal skip)
    E       = exp(sT * 1/sqrt(C))   ACT from psum; DVE 0/1-mask on the two
                                    diagonal blocks only (one [128,128] mask)
    O       = P.T @ [v|1]           3 MMs N=65 (stationary = P blocks);
                                    col 64 of each t-tile = denominator Z
    out     = O * (1/Z)             DVE strided reciprocal, ACT per-partition
                                    scale -> [128, 128] fp16, one DMA out

Software pipeline: projections run TWO iterations ahead of their scores
matmul (kT DMA slack); PE stream per iteration:
    sc(b) qkT(b+2) v(b+2) oe(b)
so batch b's softmax chain (ACT exp + DVE mask) hides under batch b+2's
projections.
"""

import numpy as np

N_EMBED = 384
HEAD_SIZE = 64
T = 256
B = 256
N_CORES = 8
B_SHARD = B // N_CORES  # 32
CC = N_EMBED // 128  # 3 contraction chunks
INV_SQRT_C = 1.0 / float(np.sqrt(N_EMBED))

_CACHE = {}

# test.py can flip these before calling kernel()
TRACE = False
LAST_RESULTS = None


def _build_program():
    import concourse.bacc as bacc
    import concourse.mybir as mybir
    import concourse.tile as tile
    from concourse import bass

    f32 = mybir.dt.float32
    f16 = mybir.dt.float16
    ts = bass.ts
    Exp = mybir.ActivationFunctionType.Exp
    Copy = mybir.ActivationFunctionType.Copy

    nc = bacc.Bacc("TRN2", target_bir_lowering=False, debug=False,
                   enable_asserts=False)

    x_d = nc.dram_tensor("x", [B_SHARD, 128, CC * T], f16, kind="ExternalInput")
    wqk_d = nc.dram_tensor("Wqk", [CC, 128, 128], f16, kind="ExternalInput")
    wv_d = nc.dram_tensor("Wv", [CC, 128, HEAD_SIZE], f16, kind="ExternalInput")
    mask_d = nc.dram_tensor("mask01", [128, 128], f16, kind="ExternalInput")
    out_d = nc.dram_tensor("out", [B_SHARD, 128, 2, HEAD_SIZE], f16,
                           kind="ExternalOutput")

    x_ap = x_d.ap()
    out_ap = out_d.ap()

    with tile.TileContext(nc) as tc:
        with (
            tc.tile_pool(name="const", bufs=1) as cpool,
            tc.tile_pool(name="xin", bufs=6) as xin_pool,
            tc.tile_pool(name="proj", bufs=4) as proj_pool,
            tc.tile_pool(name="vxp", bufs=6) as vx_pool,
            tc.tile_pool(name="soft", bufs=5) as soft_pool,
            tc.tile_pool(name="outp", bufs=4) as out_pool,
            tc.tile_pool(name="ps_qk", bufs=2, space="PSUM") as psqk_pool,
            tc.tile_pool(name="ps_v", bufs=2, space="PSUM") as psv_pool,
            tc.tile_pool(name="ps_sc", bufs=2, space="PSUM") as pssc_pool,
            tc.tile_pool(name="ps_oe", bufs=2, space="PSUM") as psoe_pool,
        ):
            # ---- constants ----
            wqk_sb, wv_sb = [], []
            for cc in range(CC):
                t_ = cpool.tile([128, 128], f16, tag=f"wqk{cc}")
                nc.sync.dma_start(t_[:], wqk_d.ap()[cc])
                wqk_sb.append(t_)
                t_ = cpool.tile([128, HEAD_SIZE], f16, tag=f"wv{cc}")
                nc.sync.dma_start(t_[:], wv_d.ap()[cc])
                wv_sb.append(t_)
            mask_sb = cpool.tile([128, 128], f16, tag="mask")
            nc.sync.dma_start(mask_sb[:], mask_d.ap())

            def load_x(b):
                # whole batch in one DMA: contiguous 1536B per partition
                t_ = xin_pool.tile([128, CC * T], f16, tag="xt")
                nc.sync.dma_start(t_[:], x_ap[b])
                return t_

            def proj_qk(xt):
                """q,k projections for one batch.
                Returns (qk_sb [128,256] f16, kT [64,256] f16)."""
                ps = psqk_pool.tile([128, T], f32, tag="ps_qk")
                for cc in range(CC):
                    nc.tensor.matmul(ps[:], wqk_sb[cc][:], xt[:, ts(cc, T)],
                                     start=(cc == 0), stop=(cc == CC - 1))
                qk = proj_pool.tile([128, T], f16, tag="qk")
                nc.vector.tensor_copy(qk[:], ps[:])
                kT = proj_pool.tile([HEAD_SIZE, T], f16, tag="kT")
                nc.sync.dma_start(kT[:], qk[HEAD_SIZE:128, :])
                return qk, kT

            def proj_v(xt):
                """v projection -> v_ext [128,130] = [v_t0 | 1 | v_t1 | 1].
                One strided DVE copy moves both v halves (GPSIMD can't read
                PSUM on TRN2); GPSIMD writes the ones columns."""
                psv = psv_pool.tile([128, 2 * HEAD_SIZE], f32, tag="ps_v")
                for tt in range(2):
                    for cc in range(CC):
                        nc.tensor.matmul(psv[:, ts(tt, HEAD_SIZE)],
                                         xt[:, cc * T + tt * 128:
                                            cc * T + (tt + 1) * 128],
                                         wv_sb[cc][:],
                                         start=(cc == 0), stop=(cc == CC - 1))
                vx = vx_pool.tile([128, 130], f16, tag="vx")
                nc.vector.tensor_copy(
                    vx[:].rearrange("p (g h) -> p g h", h=65)[:, :, 0:HEAD_SIZE],
                    psv[:].rearrange("p (g h) -> p g h", h=HEAD_SIZE))
                nc.gpsimd.memset(vx[:, HEAD_SIZE::65], 1.0)
                return vx

            def scores(qk, kT):
                """scoresT psum [128, 384]: st0 at [:, 0:256], st1 (t>=128 only)
                at [:, 256:384]."""
                ps = pssc_pool.tile([128, 384], f32, tag="ps_sc")
                nc.tensor.matmul(ps[:, 0:T], kT[:, 0:128], qk[:HEAD_SIZE, :],
                                 start=True, stop=True)
                nc.tensor.matmul(ps[:, T:384], kT[:, 128:T],
                                 qk[:HEAD_SIZE, 128:T], start=True, stop=True)
                return ps

            def softmax(sc_ps):
                """e [128,384] = exp of all three score blocks in ONE ACT op
                (ACT costs (N+352)/1.2 ns -- fixed overhead dominates small
                ops). pm [128,256]: causal-masked diagonal blocks (GpSimd,
                which is otherwise idle; DVE is loaded)."""
                e = soft_pool.tile([128, 384], f16, tag="e")
                nc.scalar.activation(e[:], sc_ps[:], Exp, scale=INV_SQRT_C)
                pm = soft_pool.tile([128, 256], f16, tag="pm")
                nc.gpsimd.tensor_mul(pm[:, 0:128], e[:, 0:128], mask_sb[:])
                nc.gpsimd.tensor_mul(pm[:, 128:256], e[:, 256:384], mask_sb[:])
                return e, pm

            def oe(e, pm, vx):
                """O psum [128, 130]: t-tile tt at [:, tt*65 : tt*65+65];
                col tt*65+64 = denominator Z."""
                ps = psoe_pool.tile([128, 130], f32, tag="ps_oe")
                nc.tensor.matmul(ps[:, 0:65], pm[:, 0:128], vx[:, 0:65],
                                 start=True, stop=True)
                nc.tensor.matmul(ps[:, 65:130], e[:, 128:T], vx[:, 0:65],
                                 start=True, stop=False)
                nc.tensor.matmul(ps[:, 65:130], pm[:, 128:256], vx[:, 65:130],
                                 start=False, stop=True)
                return ps

            def norm_store(b, ps):
                rz = out_pool.tile([128, 2], f32, tag="rz")
                # both Z columns (offsets 64 and 129) in one strided reciprocal
                nc.vector.reciprocal(rz[:], ps[:, HEAD_SIZE::65])
                o = out_pool.tile([128, 2 * HEAD_SIZE], f16, tag="o")
                # one DVE multiply for both t-tiles: [128, 2, 64] with the
                # per-(partition, tile) reciprocal broadcast along h
                nc.vector.tensor_mul(
                    o[:].rearrange("p (g h) -> p g h", h=HEAD_SIZE),
                    ps[:].rearrange("p (g h) -> p g h", h=65)[:, :, 0:HEAD_SIZE],
                    rz[:].unsqueeze(2).broadcast_to([128, 2, HEAD_SIZE]))
                nc.scalar.dma_start(
                    out_ap[b].rearrange("p tt h -> p (tt h)"), o[:])

            # ---- software-pipelined batch loop ----
            # Projections run TWO iterations ahead of their scores matmul so
            # the cross-partition kT DMA has a full iteration of slack, and
            # oe/norm run TWO iterations behind their softmax so NO PE
            # instruction ever waits on the ACT-exp -> GpSimd-mask chain
            # (PE stalls de-warm the HAM clock gate, halving the PE clock).
            # PE stream per iteration: sc(b) qkT(b+2) oe(b-2) v(b+2) --
            # oe's short-N matmuls sit after qkT's long streams so their
            # LDWEIGHTS hide under them.
            x_nat = [None] * B_SHARD
            pr, pend = {}, {}
            for j in range(min(4, B_SHARD)):
                x_nat[j] = load_x(j)
            for j in range(min(2, B_SHARD)):
                pr[j] = proj_qk(x_nat[j]) + (proj_v(x_nat[j]),)
            for b in range(B_SHARD + 2):
                if b + 4 < B_SHARD:
                    x_nat[b + 4] = load_x(b + 4)
                if b < B_SHARD:
                    qk, kT, vx = pr.pop(b)
                    sc_ps = scores(qk, kT)
                    e, pm = softmax(sc_ps)
                    pend[b] = (e, pm, vx)
                    if b + 2 < B_SHARD:
                        qk2, kT2 = proj_qk(x_nat[b + 2])
                        if b >= 2:
                            norm_store(b - 2, oe(*pend.pop(b - 2)))
                        pr[b + 2] = (qk2, kT2, proj_v(x_nat[b + 2]))
                    elif b >= 2:
                        norm_store(b - 2, oe(*pend.pop(b - 2)))
                elif b >= 2:
                    norm_store(b - 2, oe(*pend.pop(b - 2)))

    nc.compile()
    return nc


def _consts():
    # mask01[s, t] = 1 where s <= t else 0 -- shared by both diagonal blocks
    s = np.arange(128)[:, None]
    t = np.arange(128)[None, :]
    return (s <= t).astype(np.float16)


def _spot_check(out, x, Wq, Wk, Wv, batches):
    """Numpy reference for a few batches -- guards against transient device
    flakiness. The fp16 kernel's error is ~3e-3 abs; garbage is ~1e0."""
    for b in batches:
        xb = np.asarray(x[b], dtype=np.float32)
        q = xb @ Wq
        k = xb @ Wk
        v = xb @ Wv
        s = (q @ k.T) * np.float32(INV_SQRT_C)
        tmask = np.tril(np.ones((T, T), dtype=bool))
        s = np.where(tmask, s, -np.inf)
        w = np.exp(s - s.max(axis=-1, keepdims=True))
        o = (w @ v) / w.sum(axis=-1, keepdims=True)
        if np.max(np.abs(out[b] - o)) > 0.05 * max(np.max(np.abs(o)), 1e-3):
            return False
    return True


def kernel(x, Wq, Wk, Wv):
    global LAST_RESULTS
    from concourse import bass_utils

    if "nc" not in _CACHE:
        _CACHE["nc"] = _build_program()
    nc = _CACHE["nc"]

    # host-side layout prep (free): xT[b, p, cc, t] = x[b, t, cc*128 + p]
    x16 = np.asarray(x, dtype=np.float16)
    xt = np.ascontiguousarray(
        x16.transpose(0, 2, 1)               # [B, C, T]
           .reshape(B, CC, 128, T)
           .transpose(0, 2, 1, 3)            # [B, 128, CC, T]
           .reshape(B, 128, CC * T))
    # [Wq | Wk] stacked on the output dim, chunked along the contraction dim
    wqk = np.concatenate([np.asarray(Wq), np.asarray(Wk)], axis=1)
    wqk16 = np.ascontiguousarray(
        wqk.reshape(CC, 128, 2 * HEAD_SIZE), dtype=np.float16)
    wv16 = np.ascontiguousarray(
        np.asarray(Wv, dtype=np.float16).reshape(CC, 128, HEAD_SIZE))
    mask01 = _consts()

    in_maps = []
    for c in range(N_CORES):
        in_maps.append({
            "x": xt[c * B_SHARD:(c + 1) * B_SHARD],
            "Wqk": wqk16, "Wv": wv16, "mask01": mask01,
        })

    xf = np.ascontiguousarray(x, dtype=np.float32)
    Wqf = np.asarray(Wq, dtype=np.float32)
    Wkf = np.asarray(Wk, dtype=np.float32)
    Wvf = np.asarray(Wv, dtype=np.float32)
    check_batches = [c * B_SHARD for c in range(N_CORES)]
    for attempt in range(3):
        res = bass_utils.run_bass_kernel_spmd(
            nc, in_maps, core_ids=list(range(N_CORES)), trace=TRACE)
        LAST_RESULTS = res
        out = np.concatenate(
            [res.results[c]["out"].transpose(0, 2, 1, 3)
             .reshape(B_SHARD, T, HEAD_SIZE) for c in range(N_CORES)], axis=0)
        out = np.ascontiguousarray(out, dtype=np.float32)
        if _spot_check(out, xf, Wqf, Wkf, Wvf, check_batches):
            return out
    return out


# revision 23
# speedup vs baseline: 1.3414x; 1.0277x over previous
"""Trainium2 Bass kernel for a causal single-head attention block -- v5.

Same math as v4 (see kernel.py docstring) but processes batches in PAIRS to
amortize per-instruction fixed overheads measured on HW:
  - ACT: (N+352)/1.2 ns  -> one exp over both batches' scores [128, 768]
  - DVE: ~143ns fixed    -> one CAST/recip/norm/mask op per pair, strided APs
  - DMA: one x-load / kT-move / out-store per pair
  - PE:  qkT as 3 matmuls of N=512 (both batches side by side)

Per-pair layout (free dims):
  xT tile [128, (cc b2 t)]       = [128, 1536] fp16, one contiguous DMA
  qk psum/sbuf [128, (b2 t)]     = [128, 512]; kT [64, (b2 s)]
  sc psum [128, (b2 blk t)]      = [128, 768], blk = {st0-t(256), st1-t1(128)}
  e [128, 768]; pm [128, (b2 g t)] = [128, 512] masked diagonal blocks
  v psum [128, (b2 tt h)]        = [128, 256]; vx [128, (b2 tt [v|1])] = [128, 260]
  oe psum [128, (b2 tt [o|Z])]   = [128, 260]; out [128, (b2 tt h)] = [128, 256]

Pipeline (pair-granular): projections ONE pair-iteration ahead (kT DMA
slack), oe/norm ONE pair-iteration behind (softmax chain slack). PE stream
per iteration: sc(p) qkT(p+1) oe(p-1) v(p+1).
"""

import numpy as np

N_EMBED = 384
HEAD_SIZE = 64
H1 = HEAD_SIZE + 1
T = 256
B = 256
N_CORES = 8
B_SHARD = B // N_CORES  # 32
NP = B_SHARD // 2       # 16 pairs
CC = N_EMBED // 128     # 3 contraction chunks
INV_SQRT_C = 1.0 / float(np.sqrt(N_EMBED))

_CACHE = {}
TRACE = False
LAST_RESULTS = None


def _build_program():
    import concourse.bacc as bacc
    import concourse.mybir as mybir
    import concourse.tile as tile
    from concourse import bass

    f32 = mybir.dt.float32
    f16 = mybir.dt.float16
    ts = bass.ts
    Exp = mybir.ActivationFunctionType.Exp

    nc = bacc.Bacc("TRN2", target_bir_lowering=False, debug=False,
                   enable_asserts=False)

    x_d = nc.dram_tensor("x", [NP, 128, 2 * CC * T], f16, kind="ExternalInput")
    wqk_d = nc.dram_tensor("Wqk", [CC, 128, 128], f16, kind="ExternalInput")
    wv_d = nc.dram_tensor("Wv", [CC, 128, HEAD_SIZE], f16, kind="ExternalInput")
    mask_d = nc.dram_tensor("mask01", [128, 128], f16, kind="ExternalInput")
    out_d = nc.dram_tensor("out", [NP, 128, 4, HEAD_SIZE], f16,
                           kind="ExternalOutput")

    x_ap = x_d.ap()
    out_ap = out_d.ap()

    with tile.TileContext(nc) as tc:
        with (
            tc.tile_pool(name="const", bufs=1) as cpool,
            tc.tile_pool(name="xin", bufs=4) as xin_pool,
            tc.tile_pool(name="proj", bufs=3) as proj_pool,
            tc.tile_pool(name="vxp", bufs=3) as vx_pool,
            tc.tile_pool(name="soft", bufs=3) as soft_pool,
            tc.tile_pool(name="outp", bufs=3) as out_pool,
            # PSUM pools are bank-granular (2KB/partition per buf).
            # scv tiles [128, 512] hold one batch's scores [0:384] AND its v
            # projection [384:512] in one bank; 4 bufs = 2 in-flight pairs.
            tc.tile_pool(name="ps_qk", bufs=2, space="PSUM") as psqk_pool,
            tc.tile_pool(name="ps_scv", bufs=4, space="PSUM") as pssc_pool,
            tc.tile_pool(name="ps_oe", bufs=2, space="PSUM") as psoe_pool,
        ):
            # ---- constants ----
            wqk_sb, wv_sb = [], []
            for cc in range(CC):
                t_ = cpool.tile([128, 128], f16, tag=f"wqk{cc}")
                nc.sync.dma_start(t_[:], wqk_d.ap()[cc])
                wqk_sb.append(t_)
                t_ = cpool.tile([128, HEAD_SIZE], f16, tag=f"wv{cc}")
                nc.sync.dma_start(t_[:], wv_d.ap()[cc])
                wv_sb.append(t_)
            mask_sb = cpool.tile([128, 128], f16, tag="mask")
            nc.sync.dma_start(mask_sb[:], mask_d.ap())

            def load_x(p):
                t_ = xin_pool.tile([128, 2 * CC * T], f16, tag="xt")
                nc.sync.dma_start(t_[:], x_ap[p])
                return t_

            def proj_qk(xt):
                ps = psqk_pool.tile([128, 2 * T], f32, tag="ps_qk")
                for cc in range(CC):
                    nc.tensor.matmul(ps[:], wqk_sb[cc][:],
                                     xt[:, ts(cc, 2 * T)],
                                     start=(cc == 0), stop=(cc == CC - 1))
                qk = proj_pool.tile([128, 2 * T], f16, tag="qk")
                nc.vector.tensor_copy(qk[:], ps[:])
                kT = proj_pool.tile([HEAD_SIZE, 2 * T], f16, tag="kT")
                nc.sync.dma_start(kT[:], qk[HEAD_SIZE:128, :])
                return qk, kT

            def proj_v(xt):
                """v projection into scv[b2][:, 384:512]; vx [128, 260].
                The scv tiles are drawn here (one iteration ahead) and handed
                to scores()/softmax() next iteration -- the v region and the
                scores region of one bank have disjoint lifetimes."""
                scv = [pssc_pool.tile([128, 512], f32, tag="scv",
                                      name=f"scv{b2}")
                       for b2 in range(2)]
                vx = vx_pool.tile([128, 4 * 65], f16, tag="vx")
                for b2 in range(2):
                    psv = scv[b2]
                    for tt in range(2):
                        for cc in range(CC):
                            nc.tensor.matmul(
                                psv[:, 384 + tt * HEAD_SIZE:
                                    384 + (tt + 1) * HEAD_SIZE],
                                xt[:, cc * 512 + b2 * 256 + tt * 128:
                                   cc * 512 + b2 * 256 + (tt + 1) * 128],
                                wv_sb[cc][:],
                                start=(cc == 0), stop=(cc == CC - 1))
                    nc.vector.tensor_copy(
                        vx[:, b2 * 130: (b2 + 1) * 130].rearrange(
                            "p (g h) -> p g h", h=65)[:, :, 0:HEAD_SIZE],
                        psv[:, 384:512].rearrange("p (g h) -> p g h",
                                                  h=HEAD_SIZE))
                nc.gpsimd.memset(vx[:, HEAD_SIZE::65], 1.0)
                return vx, scv

            def scores(qk, kT, scv):
                for b2 in range(2):
                    ps = scv[b2]
                    nc.tensor.matmul(ps[:, 0:T],
                                     kT[:, b2 * T: b2 * T + 128],
                                     qk[:HEAD_SIZE, b2 * T: (b2 + 1) * T],
                                     start=True, stop=True)
                    nc.tensor.matmul(ps[:, T:384],
                                     kT[:, b2 * T + 128: (b2 + 1) * T],
                                     qk[:HEAD_SIZE, b2 * T + 128: (b2 + 1) * T],
                                     start=True, stop=True)

            def softmax(scv):
                e = soft_pool.tile([128, 2 * 384], f16, tag="e")
                pm = soft_pool.tile([128, 2 * 256], f16, tag="pm")
                mb = mask_sb[:].unsqueeze(1).broadcast_to([128, 2, 128])
                for b2 in range(2):
                    nc.scalar.activation(e[:, ts(b2, 384)], scv[b2][:, 0:384],
                                         Exp, scale=INV_SQRT_C)
                    nc.vector.tensor_mul(
                        pm[:, ts(b2, 256)].rearrange("p (g t) -> p g t", t=128),
                        e[:, ts(b2, 384)].rearrange(
                            "p (g t) -> p g t", t=128)[:, 0::2, :],
                        mb)
                return e, pm

            def oe(e, pm, vx):
                ps = psoe_pool.tile([128, 4 * 65], f32, tag="ps_oe")
                for b2 in range(2):
                    o0 = b2 * 130
                    nc.tensor.matmul(ps[:, o0: o0 + 65],
                                     pm[:, b2 * 256: b2 * 256 + 128],
                                     vx[:, o0: o0 + 65],
                                     start=True, stop=True)
                    nc.tensor.matmul(ps[:, o0 + 65: o0 + 130],
                                     e[:, b2 * 384 + 128: b2 * 384 + 256],
                                     vx[:, o0: o0 + 65],
                                     start=True, stop=False)
                    nc.tensor.matmul(ps[:, o0 + 65: o0 + 130],
                                     pm[:, b2 * 256 + 128: (b2 + 1) * 256],
                                     vx[:, o0 + 65: o0 + 130],
                                     start=False, stop=True)
                return ps

            def norm_store(p, ps):
                rz = out_pool.tile([128, 4], f32, tag="rz")
                nc.vector.reciprocal(rz[:], ps[:, HEAD_SIZE::65])
                o = out_pool.tile([128, 4 * HEAD_SIZE], f16, tag="o")
                nc.vector.tensor_mul(
                    o[:].rearrange("p (g h) -> p g h", h=HEAD_SIZE),
                    ps[:].rearrange("p (g h) -> p g h", h=65)[:, :, 0:HEAD_SIZE],
                    rz[:].unsqueeze(2).broadcast_to([128, 4, HEAD_SIZE]))
                nc.scalar.dma_start(
                    out_ap[p].rearrange("p g h -> p (g h)"), o[:])

            # ---- software-pipelined pair loop ----
            x_nat = [None] * NP
            pr, pend = {}, {}
            for j in range(min(3, NP)):
                x_nat[j] = load_x(j)
            pr[0] = proj_qk(x_nat[0]) + proj_v(x_nat[0])
            for p in range(NP + 1):
                if p + 3 < NP:
                    x_nat[p + 3] = load_x(p + 3)
                if p < NP:
                    qk, kT, vx, scv = pr.pop(p)
                    scores(qk, kT, scv)
                    e, pm = softmax(scv)
                    if p + 1 < NP:
                        nxt = proj_qk(x_nat[p + 1])
                    if p >= 1:
                        norm_store(p - 1, oe(*pend.pop(p - 1)))
                    if p + 1 < NP:
                        pr[p + 1] = nxt + proj_v(x_nat[p + 1])
                    pend[p] = (e, pm, vx)
                else:
                    norm_store(p - 1, oe(*pend.pop(p - 1)))

    nc.compile()
    return nc


def _consts():
    s = np.arange(128)[:, None]
    t = np.arange(128)[None, :]
    return (s <= t).astype(np.float16)


def _spot_check(out, x, Wq, Wk, Wv, batches):
    for b in batches:
        xb = np.asarray(x[b], dtype=np.float32)
        q = xb @ Wq
        k = xb @ Wk
        v = xb @ Wv
        s = (q @ k.T) * np.float32(INV_SQRT_C)
        tmask = np.tril(np.ones((T, T), dtype=bool))
        s = np.where(tmask, s, -np.inf)
        w = np.exp(s - s.max(axis=-1, keepdims=True))
        o = (w @ v) / w.sum(axis=-1, keepdims=True)
        if np.max(np.abs(out[b] - o)) > 0.05 * max(np.max(np.abs(o)), 1e-3):
            return False
    return True


def kernel(x, Wq, Wk, Wv):
    global LAST_RESULTS
    from concourse import bass_utils

    if "nc" not in _CACHE:
        _CACHE["nc"] = _build_program()
    nc = _CACHE["nc"]

    # host-side layout prep (free):
    # xt[pair, p, cc, b2, t] = x[2*pair + b2, t, cc*128 + p]
    x16 = np.asarray(x, dtype=np.float16)
    xt = np.ascontiguousarray(
        x16.transpose(0, 2, 1)                    # [B, C, T]
           .reshape(B // 2, 2, CC, 128, T)        # [bp, b2, cc, p, t]
           .transpose(0, 3, 2, 1, 4)              # [bp, p, cc, b2, t]
           .reshape(B // 2, 128, 2 * CC * T))
    wqk = np.concatenate([np.asarray(Wq), np.asarray(Wk)], axis=1)
    wqk16 = np.ascontiguousarray(
        wqk.reshape(CC, 128, 2 * HEAD_SIZE), dtype=np.float16)
    wv16 = np.ascontiguousarray(
        np.asarray(Wv, dtype=np.float16).reshape(CC, 128, HEAD_SIZE))
    mask01 = _consts()

    in_maps = []
    for c in range(N_CORES):
        in_maps.append({
            "x": xt[c * NP:(c + 1) * NP],
            "Wqk": wqk16, "Wv": wv16, "mask01": mask01,
        })

    xf = np.ascontiguousarray(x, dtype=np.float32)
    Wqf = np.asarray(Wq, dtype=np.float32)
    Wkf = np.asarray(Wk, dtype=np.float32)
    Wvf = np.asarray(Wv, dtype=np.float32)
    check_batches = [c * B_SHARD for c in range(N_CORES)]
    for attempt in range(3):
        res = bass_utils.run_bass_kernel_spmd(
            nc, in_maps, core_ids=list(range(N_CORES)), trace=TRACE)
        LAST_RESULTS = res
        # out[pair, p, (b2 tt), h] -> [B, T, H]
        out = np.concatenate(
            [res.results[c]["out"].reshape(NP, 128, 2, 2, HEAD_SIZE)
             .transpose(0, 2, 3, 1, 4).reshape(B_SHARD, T, HEAD_SIZE)
             for c in range(N_CORES)], axis=0)
        out = np.ascontiguousarray(out, dtype=np.float32)
        if _spot_check(out, xf, Wqf, Wkf, Wvf, check_batches):
            return out
    return out


# revision 26
# speedup vs baseline: 1.4235x; 1.0612x over previous
"""Trainium2 Bass kernel for a causal single-head attention block -- v5.

Same math as v4 (see kernel.py docstring) but processes batches in PAIRS to
amortize per-instruction fixed overheads measured on HW:
  - ACT: (N+352)/1.2 ns  -> one exp over both batches' scores [128, 768]
  - DVE: ~143ns fixed    -> one CAST/recip/norm/mask op per pair, strided APs
  - DMA: one x-load / kT-move / out-store per pair
  - PE:  qkT as 3 matmuls of N=512 (both batches side by side)

Per-pair layout (free dims):
  xT tile [128, (cc b2 t)]       = [128, 1536] fp16, one contiguous DMA
  qk psum/sbuf [128, (b2 t)]     = [128, 512]; kT [64, (b2 s)]
  sc psum [128, (b2 blk t)]      = [128, 768], blk = {st0-t(256), st1-t1(128)}
  e [128, 768]; pm [128, (b2 g t)] = [128, 512] masked diagonal blocks
  v psum [128, (b2 tt h)]        = [128, 256]; vx [128, (b2 tt [v|1])] = [128, 260]
  oe psum [128, (b2 tt [o|Z])]   = [128, 260]; out [128, (b2 tt h)] = [128, 256]

Pipeline (pair-granular): projections ONE pair-iteration ahead (kT DMA
slack), oe/norm ONE pair-iteration behind (softmax chain slack). PE stream
per iteration: sc(p) qkT(p+1) oe(p-1) v(p+1).
"""

import numpy as np

N_EMBED = 384
HEAD_SIZE = 64
H1 = HEAD_SIZE + 1
T = 256
B = 256
N_CORES = 8
B_SHARD = B // N_CORES  # 32
NP = B_SHARD // 2       # 16 pairs
CC = N_EMBED // 128     # 3 contraction chunks
INV_SQRT_C = 1.0 / float(np.sqrt(N_EMBED))

_CACHE = {}
TRACE = False
LAST_RESULTS = None


def _build_program():
    import concourse.bacc as bacc
    import concourse.mybir as mybir
    import concourse.tile as tile
    from concourse import bass

    f32 = mybir.dt.float32
    f16 = mybir.dt.float16
    ts = bass.ts
    Exp = mybir.ActivationFunctionType.Exp

    nc = bacc.Bacc("TRN2", target_bir_lowering=False, debug=False,
                   enable_asserts=False)

    x_d = nc.dram_tensor("x", [NP, 128, 2 * CC * T], f16, kind="ExternalInput")
    wqk_d = nc.dram_tensor("Wqk", [CC, 128, 128], f16, kind="ExternalInput")
    wv_d = nc.dram_tensor("Wv", [CC, 128, HEAD_SIZE], f16, kind="ExternalInput")
    mask_d = nc.dram_tensor("mask01", [128, 128], f16, kind="ExternalInput")
    out_d = nc.dram_tensor("out", [NP, 128, 4, HEAD_SIZE], f16,
                           kind="ExternalOutput")

    x_ap = x_d.ap()
    out_ap = out_d.ap()

    with tile.TileContext(nc) as tc:
        with (
            tc.tile_pool(name="const", bufs=1) as cpool,
            tc.tile_pool(name="xin", bufs=5) as xin_pool,
            tc.tile_pool(name="proj", bufs=5) as proj_pool,
            tc.tile_pool(name="vxp", bufs=3) as vx_pool,
            tc.tile_pool(name="soft", bufs=3) as soft_pool,
            tc.tile_pool(name="outp", bufs=3) as out_pool,
            # PSUM pools are bank-granular (2KB/partition per buf).
            # scv tiles [128, 512] hold one batch's scores [0:384] AND its v
            # projection [384:512] in one bank; 4 bufs = 2 in-flight pairs.
            tc.tile_pool(name="ps_qk", bufs=2, space="PSUM") as psqk_pool,
            tc.tile_pool(name="ps_scv", bufs=4, space="PSUM") as pssc_pool,
            tc.tile_pool(name="ps_oe", bufs=2, space="PSUM") as psoe_pool,
        ):
            # ---- constants ----
            wqk_sb, wv_sb = [], []
            for cc in range(CC):
                t_ = cpool.tile([128, 128], f16, tag=f"wqk{cc}")
                nc.sync.dma_start(t_[:], wqk_d.ap()[cc])
                wqk_sb.append(t_)
                t_ = cpool.tile([128, HEAD_SIZE], f16, tag=f"wv{cc}")
                nc.sync.dma_start(t_[:], wv_d.ap()[cc])
                wv_sb.append(t_)
            mask_sb = cpool.tile([128, 128], f16, tag="mask")
            nc.sync.dma_start(mask_sb[:], mask_d.ap())

            def load_x(p):
                t_ = xin_pool.tile([128, 2 * CC * T], f16, tag="xt")
                nc.sync.dma_start(t_[:], x_ap[p])
                return t_

            def proj_qk(xt):
                ps = psqk_pool.tile([128, 2 * T], f32, tag="ps_qk")
                for cc in range(CC):
                    nc.tensor.matmul(ps[:], wqk_sb[cc][:],
                                     xt[:, ts(cc, 2 * T)],
                                     start=(cc == 0), stop=(cc == CC - 1))
                qk = proj_pool.tile([128, 2 * T], f16, tag="qk")
                nc.vector.tensor_copy(qk[:], ps[:])
                kT = proj_pool.tile([HEAD_SIZE, 2 * T], f16, tag="kT")
                nc.sync.dma_start(kT[:], qk[HEAD_SIZE:128, :])
                return qk, kT

            def proj_v(xt):
                """v projection into scv[b2][:, 384:512]; vx [128, 260].
                The scv tiles are drawn here (one iteration ahead) and handed
                to scores()/softmax() next iteration -- the v region and the
                scores region of one bank have disjoint lifetimes."""
                scv = [pssc_pool.tile([128, 512], f32, tag="scv",
                                      name=f"scv{b2}")
                       for b2 in range(2)]
                vx = vx_pool.tile([128, 4 * 65], f16, tag="vx")
                for b2 in range(2):
                    psv = scv[b2]
                    for tt in range(2):
                        for cc in range(CC):
                            nc.tensor.matmul(
                                psv[:, 384 + tt * HEAD_SIZE:
                                    384 + (tt + 1) * HEAD_SIZE],
                                xt[:, cc * 512 + b2 * 256 + tt * 128:
                                   cc * 512 + b2 * 256 + (tt + 1) * 128],
                                wv_sb[cc][:],
                                start=(cc == 0), stop=(cc == CC - 1))
                    nc.vector.tensor_copy(
                        vx[:, b2 * 130: (b2 + 1) * 130].rearrange(
                            "p (g h) -> p g h", h=65)[:, :, 0:HEAD_SIZE],
                        psv[:, 384:512].rearrange("p (g h) -> p g h",
                                                  h=HEAD_SIZE))
                nc.gpsimd.memset(vx[:, HEAD_SIZE::65], 1.0)
                return vx, scv

            def scores(qk, kT, scv):
                for b2 in range(2):
                    ps = scv[b2]
                    nc.tensor.matmul(ps[:, 0:T],
                                     kT[:, b2 * T: b2 * T + 128],
                                     qk[:HEAD_SIZE, b2 * T: (b2 + 1) * T],
                                     start=True, stop=True)
                    nc.tensor.matmul(ps[:, T:384],
                                     kT[:, b2 * T + 128: (b2 + 1) * T],
                                     qk[:HEAD_SIZE, b2 * T + 128: (b2 + 1) * T],
                                     start=True, stop=True)

            def softmax(scv):
                e = soft_pool.tile([128, 2 * 384], f16, tag="e")
                pm = soft_pool.tile([128, 2 * 256], f16, tag="pm")
                mb = mask_sb[:].unsqueeze(1).broadcast_to([128, 2, 128])
                for b2 in range(2):
                    nc.scalar.activation(e[:, ts(b2, 384)], scv[b2][:, 0:384],
                                         Exp, scale=INV_SQRT_C)
                    # one mask on DVE, one on the otherwise-idle GpSimd
                    eng = nc.vector if b2 else nc.gpsimd
                    eng.tensor_mul(
                        pm[:, ts(b2, 256)].rearrange("p (g t) -> p g t", t=128),
                        e[:, ts(b2, 384)].rearrange(
                            "p (g t) -> p g t", t=128)[:, 0::2, :],
                        mb)
                return e, pm

            def oe(e, pm, vx):
                ps = psoe_pool.tile([128, 4 * 65], f32, tag="ps_oe")
                for b2 in range(2):
                    o0 = b2 * 130
                    nc.tensor.matmul(ps[:, o0: o0 + 65],
                                     pm[:, b2 * 256: b2 * 256 + 128],
                                     vx[:, o0: o0 + 65],
                                     start=True, stop=True)
                    nc.tensor.matmul(ps[:, o0 + 65: o0 + 130],
                                     e[:, b2 * 384 + 128: b2 * 384 + 256],
                                     vx[:, o0: o0 + 65],
                                     start=True, stop=False)
                    nc.tensor.matmul(ps[:, o0 + 65: o0 + 130],
                                     pm[:, b2 * 256 + 128: (b2 + 1) * 256],
                                     vx[:, o0 + 65: o0 + 130],
                                     start=False, stop=True)
                return ps

            def norm_store(p, ps):
                rz = out_pool.tile([128, 4], f32, tag="rz")
                nc.vector.reciprocal(rz[:], ps[:, HEAD_SIZE::65])
                o = out_pool.tile([128, 4 * HEAD_SIZE], f16, tag="o")
                nc.vector.tensor_mul(
                    o[:].rearrange("p (g h) -> p g h", h=HEAD_SIZE),
                    ps[:].rearrange("p (g h) -> p g h", h=65)[:, :, 0:HEAD_SIZE],
                    rz[:].unsqueeze(2).broadcast_to([128, 4, HEAD_SIZE]))
                nc.scalar.dma_start(
                    out_ap[p].rearrange("p g h -> p (g h)"), o[:])

            # ---- software-pipelined pair loop ----
            # proj_qk runs TWO pair-iterations ahead (the qk CAST lands late
            # on the saturated DVE, and the kT DMA needs ~0.6us HWDGE latency
            # after it); proj_v only ONE ahead (its scv psum tiles must stay
            # within 4 banks). oe/norm one pair behind the softmax.
            x_nat = [None] * NP
            prqk, prv, pend = {}, {}, {}
            for j in range(min(4, NP)):
                x_nat[j] = load_x(j)
            for j in range(min(2, NP)):
                prqk[j] = proj_qk(x_nat[j])
            prv[0] = proj_v(x_nat[0])
            for p in range(NP + 1):
                if p + 4 < NP:
                    x_nat[p + 4] = load_x(p + 4)
                if p < NP:
                    qk, kT = prqk.pop(p)
                    vx, scv = prv.pop(p)
                    scores(qk, kT, scv)
                    e, pm = softmax(scv)
                    if p + 2 < NP:
                        prqk[p + 2] = proj_qk(x_nat[p + 2])
                    if p >= 1:
                        norm_store(p - 1, oe(*pend.pop(p - 1)))
                    if p + 1 < NP:
                        prv[p + 1] = proj_v(x_nat[p + 1])
                    pend[p] = (e, pm, vx)
                else:
                    norm_store(p - 1, oe(*pend.pop(p - 1)))

    nc.compile()
    return nc


def _consts():
    s = np.arange(128)[:, None]
    t = np.arange(128)[None, :]
    return (s <= t).astype(np.float16)


def _spot_check(out, x, Wq, Wk, Wv, batches):
    for b in batches:
        xb = np.asarray(x[b], dtype=np.float32)
        q = xb @ Wq
        k = xb @ Wk
        v = xb @ Wv
        s = (q @ k.T) * np.float32(INV_SQRT_C)
        tmask = np.tril(np.ones((T, T), dtype=bool))
        s = np.where(tmask, s, -np.inf)
        w = np.exp(s - s.max(axis=-1, keepdims=True))
        o = (w @ v) / w.sum(axis=-1, keepdims=True)
        if np.max(np.abs(out[b] - o)) > 0.05 * max(np.max(np.abs(o)), 1e-3):
            return False
    return True


def kernel(x, Wq, Wk, Wv):
    global LAST_RESULTS
    from concourse import bass_utils

    if "nc" not in _CACHE:
        _CACHE["nc"] = _build_program()
    nc = _CACHE["nc"]

    # host-side layout prep (free):
    # xt[pair, p, cc, b2, t] = x[2*pair + b2, t, cc*128 + p]
    x16 = np.asarray(x, dtype=np.float16)
    xt = np.ascontiguousarray(
        x16.transpose(0, 2, 1)                    # [B, C, T]
           .reshape(B // 2, 2, CC, 128, T)        # [bp, b2, cc, p, t]
           .transpose(0, 3, 2, 1, 4)              # [bp, p, cc, b2, t]
           .reshape(B // 2, 128, 2 * CC * T))
    wqk = np.concatenate([np.asarray(Wq), np.asarray(Wk)], axis=1)
    wqk16 = np.ascontiguousarray(
        wqk.reshape(CC, 128, 2 * HEAD_SIZE), dtype=np.float16)
    wv16 = np.ascontiguousarray(
        np.asarray(Wv, dtype=np.float16).reshape(CC, 128, HEAD_SIZE))
    mask01 = _consts()

    in_maps = []
    for c in range(N_CORES):
        in_maps.append({
            "x": xt[c * NP:(c + 1) * NP],
            "Wqk": wqk16, "Wv": wv16, "mask01": mask01,
        })

    xf = np.ascontiguousarray(x, dtype=np.float32)
    Wqf = np.asarray(Wq, dtype=np.float32)
    Wkf = np.asarray(Wk, dtype=np.float32)
    Wvf = np.asarray(Wv, dtype=np.float32)
    check_batches = [c * B_SHARD for c in range(N_CORES)]
    for attempt in range(3):
        res = bass_utils.run_bass_kernel_spmd(
            nc, in_maps, core_ids=list(range(N_CORES)), trace=TRACE)
        LAST_RESULTS = res
        # out[pair, p, (b2 tt), h] -> [B, T, H]
        out = np.concatenate(
            [res.results[c]["out"].reshape(NP, 128, 2, 2, HEAD_SIZE)
             .transpose(0, 2, 3, 1, 4).reshape(B_SHARD, T, HEAD_SIZE)
             for c in range(N_CORES)], axis=0)
        out = np.ascontiguousarray(out, dtype=np.float32)
        if _spot_check(out, xf, Wqf, Wkf, Wvf, check_batches):
            return out
    return out
